# revision 38
# baseline (speedup 1.0000x reference)
"""AttentionBlock (ChannelNorm + MHA + proj + residual) Trainium2 Bass kernel.

Sharding: 8 cores = 4 batches x 2 head-groups. Core c handles batch c//2 and
heads [4*(c%2), 4*(c%2)+4). Each core computes LayerNorm + its slice of the
QKV projection + attention for its 4 heads + a partial proj_out contraction.
The host sums the two partials per batch and adds proj bias + residual.

All matmuls run in float32r (fast fp32 mode, ~1.5e-4 rel err). The whole
device pipeline works in a channels-on-partitions [C, L] layout so no
transposes are needed anywhere:
  - LN stats (sum, sum-sq over C) via ones-vector matmuls, rstd via
    reciprocal_approx_fast, per-position stats broadcast across partitions
    with the Pool partition_broadcast instruction.
  - q^T,k^T come out of the QKV GEMM as [d, L]; v as [L, d] — exactly the
    layouts the attention matmuls need.
  - scores are computed transposed (s^T[lk, lq]), softmax denominator comes
    free from a ones-column appended to v (M=65 matmul), normalization is
    folded in after the o^T accumulation.

Host pre-lays every tensor out in its exact SBUF shape so each DMA is one
contiguous copy.
"""
import numpy as np

import concourse.bass as bass
import concourse.mybir as mybir
import concourse.tile as tile
from concourse import bacc
from concourse.bass_utils import run_bass_kernel_spmd

F32 = mybir.dt.float32
F32R = mybir.dt.float32r

B, C, L, H = 4, 512, 2048, 8
DH = C // H          # 64
G = 2                # head groups (cores per batch)
HPC = H // G         # 4 heads per core
P = 128
KC = C // P          # 4 contraction chunks
NSTRIP = 4
STRIP = L // NSTRIP  # 512
LCH = L // P         # 16 l-chunks
SCALE = DH ** -0.5
EPS = 1e-5
ALU = mybir.AluOpType
ACTF = mybir.ActivationFunctionType


def build_nc():
    nc = bacc.Bacc()
    x_d = nc.dram_tensor("x_sh", [NSTRIP, P, KC, STRIP], F32R, kind="ExternalInput")
    wqk_d = nc.dram_tensor("wqkT", [P, KC, 2 * HPC * DH], F32R, kind="ExternalInput")
    wv_d = nc.dram_tensor("wvT", [P, KC, HPC * DH], F32R, kind="ExternalInput")
    wp_d = nc.dram_tensor("wprojT", [P, 2, C], F32R, kind="ExternalInput")
    bqk_d = nc.dram_tensor("bqk", [P, 4], F32, kind="ExternalInput")
    wsum_d = nc.dram_tensor("wsum", [P, 4], F32, kind="ExternalInput")
    wvsum_d = nc.dram_tensor("wvsum", [1, HPC * DH], F32, kind="ExternalInput")
    bv_d = nc.dram_tensor("bv", [1, HPC * DH], F32, kind="ExternalInput")
    vones_d = nc.dram_tensor("vones", [P, LCH * HPC], F32R, kind="ExternalInput")
    out_d = nc.dram_tensor("out_part", [NSTRIP, P, 4, STRIP], F32,
                           kind="ExternalOutput")
    # DRAM bounce buffers for partition-broadcasts of per-position vectors
    rstd_d = nc.dram_tensor("rstd_scr", [NSTRIP, STRIP], F32)
    murstd_d = nc.dram_tensor("murstd_scr", [NSTRIP, STRIP], F32)
    rz_d = nc.dram_tensor("rz_scr", [HPC, 4, STRIP], F32)

    with tile.TileContext(nc) as tc:
        with (
            tc.tile_pool(name="persist", bufs=1) as pp,
            tc.tile_pool(name="small", bufs=4) as sp,
        ):
            # ---- persistent tiles ----
            ones_sb = pp.tile([P, 1], F32R)
            wqk_sb = pp.tile([P, KC, 2 * HPC * DH], F32R)    # [128,4,512]
            wv_sb = pp.tile([P, KC, HPC * DH], F32R)         # [128,4,256]
            wp_sb = pp.tile([P, 2, C], F32R)                 # [128,2,512]
            bqk_sb = pp.tile([P, 4], F32)
            wsum_sb = pp.tile([P, 4], F32)
            wvsum_b = pp.tile([P, HPC * DH], F32)
            bvb_sb = pp.tile([P, HPC * DH], F32)             # broadcast v bias
            qkT_sb = pp.tile([P, 4, L], F32R)                # q^T,k^T [c_out,l]
            v_sb = pp.tile([P, LCH, HPC, DH + 1], F32R)      # v + ones col
            onT_sb = pp.tile([P, 2, L], F32R)                # normalized o^T
            eps_sb = sp.tile([NSTRIP, 1], F32)

            nc.sync.dma_start(ones_sb[:], vones_d[:, 0:1])
            nc.vector.memset(eps_sb[:], EPS)

            # ================= phase A: LN stats + QKV GEMMs =================
            with (
                tc.tile_pool(name="xa", bufs=4) as xa,
                tc.tile_pool(name="x2a", bufs=1) as x2a,
                tc.tile_pool(name="stats", bufs=1) as st,
                tc.tile_pool(name="bcst", bufs=2) as bc,
                tc.tile_pool(name="ep2", bufs=3) as ep2,
                tc.tile_pool(name="psumA", bufs=2, space="PSUM") as psA,
            ):
                x_tiles = []
                for s in range(NSTRIP):
                    x_sb = xa.tile([P, KC, STRIP], F32R, tag="x", name=f"x{s}")
                    nc.sync.dma_start(x_sb[:], x_d[s])
                    x_tiles.append(x_sb)
                nc.sync.dma_start(wqk_sb[:], wqk_d[:])
                nc.sync.dma_start(wv_sb[:], wv_d[:])
                nc.sync.dma_start(wp_sb[:], wp_d[:])
                nc.sync.dma_start(bqk_sb[:], bqk_d[:])
                nc.sync.dma_start(wsum_sb[:], wsum_d[:])
                nc.sync.dma_start(
                    wvsum_b[:], wvsum_d[0:1, :].partition_broadcast(P).opt())
                nc.sync.dma_start(bvb_sb[:], bv_d[0:1, :].partition_broadcast(P).opt())
                nc.sync.dma_start(
                    v_sb[:, :, :, DH:DH + 1],
                    vones_d.rearrange("p (lc h one) -> p lc h one", h=HPC, one=1),
                )

                # ---- stats matmuls for every strip (PE never blocks) ----
                sum4_sb = st.tile([NSTRIP, STRIP], F32, tag="sum4")
                sq4_sb = st.tile([NSTRIP, STRIP], F32, tag="sq4")
                for s in range(NSTRIP):
                    x_sb = x_tiles[s]
                    x2 = x2a.tile([P, KC, STRIP], F32R, tag="x2",
                                  name=f"x2_{s}")
                    nc.scalar.activation(x2[:], x_sb.bitcast(F32)[:],
                                         ACTF.Square)
                    ps_sum = psA.tile([1, STRIP], F32, tag="stat_sum",
                                      name=f"psum{s}")
                    ps_sq = psA.tile([1, STRIP], F32, tag="stat_sq",
                                     name=f"psq{s}")
                    for kc in range(KC):
                        nc.tensor.matmul(
                            ps_sum[:], ones_sb[:], x_sb[:, kc, :],
                            start=(kc == 0), stop=(kc == KC - 1),
                        )
                    for kc in range(KC):
                        nc.tensor.matmul(
                            ps_sq[:], ones_sb[:], x2[:, kc, :],
                            start=(kc == 0), stop=(kc == KC - 1),
                        )
                    scr_a = st.tile([1, STRIP], F32, tag="scr_a", bufs=2,
                                    name=f"scr_a{s}")
                    scr_b = st.tile([1, STRIP], F32, tag="scr_b", bufs=2,
                                    name=f"scr_b{s}")
                    nc.vector.tensor_copy(scr_a[:], ps_sum[:])
                    nc.vector.tensor_copy(scr_b[:], ps_sq[:])
                    nc.gpsimd.dma_start(sum4_sb[s:s + 1, :], scr_a[:])
                    nc.gpsimd.dma_start(sq4_sb[s:s + 1, :], scr_b[:])

                # ---- one stats chain for all strips on [4, 512] ----
                mu = st.tile([NSTRIP, STRIP], F32, tag="mu")
                t2 = st.tile([NSTRIP, STRIP], F32, tag="t2")
                var = st.tile([NSTRIP, STRIP], F32, tag="var")
                rstd = st.tile([NSTRIP, STRIP], F32, tag="rstd")
                murstd = st.tile([NSTRIP, STRIP], F32, tag="murstd")
                nc.vector.tensor_scalar_mul(mu[:], sum4_sb[:], 1.0 / C)
                nc.vector.tensor_mul(t2[:], mu[:], mu[:])
                nc.vector.scalar_tensor_tensor(
                    var[:], sq4_sb[:], 1.0 / C, t2[:],
                    op0=ALU.mult, op1=ALU.subtract,
                )
                nc.scalar.activation(var[:], var[:], ACTF.Sqrt, bias=eps_sb[:])
                nc.vector.reciprocal(rstd[:], var[:])
                nc.vector.tensor_mul(murstd[:], mu[:], rstd[:])
                nc.gpsimd.dma_start(rstd_d[:], rstd[:])
                nc.gpsimd.dma_start(murstd_d[:], murstd[:])

                # ---- GEMMs run on RAW x; LN is folded into the epilogue:
                # qkT = rstd*(W^T x) - murstd*rowsum(W) + bias  (per position)
                for s in range(NSTRIP):
                    ls = bass.ts(s, STRIP)
                    x_sb = x_tiles[s]
                    rstd_b = bc.tile([P, STRIP], F32, tag="rstd_b")
                    murstd_b = bc.tile([P, STRIP], F32, tag="murstd_b")
                    nc.gpsimd.dma_start(
                        rstd_b[:], rstd_d[s:s + 1, :].partition_broadcast(P).opt())
                    nc.gpsimd.dma_start(
                        murstd_b[:],
                        murstd_d[s:s + 1, :].partition_broadcast(P).opt())
                    # per-position stats as [128, lc] columns for the v epilogue
                    rsc = bc.tile([P, STRIP // P], F32, tag="rsc")
                    msc = bc.tile([P, STRIP // P], F32, tag="msc")
                    nc.gpsimd.dma_start(
                        rsc[:], rstd_d[s:s + 1, :].rearrange(
                            "one (lc p) -> p (one lc)", p=P))
                    nc.gpsimd.dma_start(
                        msc[:], murstd_d[s:s + 1, :].rearrange(
                            "one (lc p) -> p (one lc)", p=P))

                    # q^T,k^T GEMM: out [c_out, l]
                    for mc in range(4):
                        pqk = psA.tile([P, STRIP], F32, tag="qk")
                        for kc in range(KC):
                            nc.tensor.matmul(
                                pqk[:], wqk_sb[:, kc, bass.ts(mc, P)],
                                x_sb[:, kc, :],
                                start=(kc == 0), stop=(kc == KC - 1),
                            )
                        tq = ep2.tile([P, STRIP], F32, tag="tq")
                        nc.gpsimd.tensor_scalar(
                            tq[:], murstd_b[:],
                            scalar1=wsum_sb[:, mc:mc + 1],
                            scalar2=bqk_sb[:, mc:mc + 1],
                            op0=ALU.mult, op1=ALU.subtract,
                        )
                        dst = qkT_sb[:, mc, ls]
                        nc.vector.tensor_mul(dst, pqk[:], rstd_b[:])
                        nc.vector.tensor_sub(dst, dst.bitcast(F32), tq[:])

                    # v GEMM: out [l, d]
                    for lc in range(STRIP // P):
                        lg = s * (STRIP // P) + lc
                        pv = psA.tile([P, HPC * DH], F32, tag="v")
                        for kc in range(KC):
                            nc.tensor.matmul(
                                pv[:], x_sb[:, kc, bass.ts(lc, P)],
                                wv_sb[:, kc, :],
                                start=(kc == 0), stop=(kc == KC - 1),
                            )
                        tv = ep2.tile([P, HPC, DH], F32, tag="tv")
                        nc.gpsimd.tensor_scalar_mul(
                            tv[:], wvsum_b.rearrange("p (h d) -> p h d", h=HPC),
                            msc[:, lc:lc + 1],
                        )
                        nc.gpsimd.tensor_sub(
                            tv[:], tv[:],
                            bvb_sb.rearrange("p (h d) -> p h d", h=HPC),
                        )
                        dst = v_sb[:, lg, :, 0:DH]
                        nc.vector.tensor_scalar_mul(
                            dst, pv.rearrange("p (h d) -> p h d", h=HPC),
                            rsc[:, lc:lc + 1],
                        )
                        nc.vector.tensor_sub(dst, dst.bitcast(F32), tv[:])

            # ================= phase B: attention per head =================
            with (
                tc.tile_pool(name="expp", bufs=4) as ep,
                tc.tile_pool(name="rdout", bufs=2) as ro,
                tc.tile_pool(name="psumB", bufs=2, space="PSUM") as psB,
                tc.tile_pool(name="psumO", bufs=4, space="PSUM") as psO,
            ):
                for h in range(HPC):
                    po = (h % 2) * DH
                    qT = qkT_sb[po:po + DH, h // 2, :]
                    kT = qkT_sb[po:po + DH, 2 + h // 2, :]
                    oT = [psO.tile([DH + 1, STRIP], F32, tag="oT", name=f"oT{h}_{i}")
                          for i in range(4)]
                    for lk in range(LCH):
                        ex = ep.tile([P, L], F32R, tag="expT")
                        for half in range(2):
                            pst = psB.tile([P, 1024], F32, tag="sT")
                            for q2 in range(2):
                                nc.tensor.matmul(
                                    pst[:, bass.ts(q2, 512)],
                                    kT[:, bass.ts(lk, P)],
                                    qT[:, bass.ds(half * 1024 + q2 * 512, 512)],
                                    start=True, stop=True,
                                )
                            nc.scalar.activation(
                                ex[:, bass.ts(half, 1024)], pst[:],
                                ACTF.Exp, scale=SCALE,
                            )
                        for s in range(4):
                            nc.tensor.matmul(
                                oT[s][:], v_sb[:, lk, h, :], ex[:, bass.ts(s, STRIP)],
                                start=(lk == 0), stop=(lk == LCH - 1),
                            )
                    # normalize: onT[d, l] = oT[d, l] / Z[l]; one batched recip
                    zrow = ro.tile([4, STRIP], F32, tag="zrow")
                    rz4 = ro.tile([4, STRIP], F32, tag="rz4")
                    for s in range(4):
                        zscr = ro.tile([1, STRIP], F32, tag="zscr",
                                       name=f"zscr{h}_{s}")
                        nc.vector.tensor_copy(zscr[:], oT[s][DH:DH + 1, :])
                        nc.sync.dma_start(zrow[s:s + 1, :], zscr[:])
                    nc.vector.reciprocal(rz4[:], zrow[:])
                    nc.sync.dma_start(rz_d[h], rz4[:])
                    for s in range(4):
                        rz_b = ro.tile([DH, STRIP], F32, tag="rz_b", bufs=4)
                        nc.sync.dma_start(
                            rz_b[:],
                            rz_d[h, s:s + 1, :].partition_broadcast(DH).opt())
                        nc.vector.tensor_mul(
                            onT_sb[po:po + DH, h // 2, bass.ts(s, STRIP)],
                            oT[s][0:DH, :], rz_b[:],
                        )

            # ================= phase C: proj partial =================
            with (
                tc.tile_pool(name="outp", bufs=2) as op_,
                tc.tile_pool(name="psumC", bufs=2, space="PSUM") as psC,
            ):
                for s in range(NSTRIP):
                    ls = bass.ts(s, STRIP)
                    ot = op_.tile([P, 4, STRIP], F32, tag="out")
                    for mc in range(4):
                        ppj = psC.tile([P, STRIP], F32, tag="proj")
                        for kc in range(2):
                            nc.tensor.matmul(
                                ppj[:], wp_sb[:, kc, bass.ts(mc, P)],
                                onT_sb[:, kc, ls],
                                start=(kc == 0), stop=(kc == 1),
                            )
                        nc.vector.tensor_copy(ot[:, mc, :], ppj[:])
                    nc.sync.dma_start(out_d[s], ot[:])

    nc.compile()
    return nc


_NC = None


def _get_nc():
    global _NC
    if _NC is None:
        _NC = build_nc()
    return _NC


def make_core_inputs(x, ln_gamma, ln_beta, w_qkv, b_qkv, w_proj, b_proj):
    """Host-side shard prep. Folds ln_gamma/ln_beta into the QKV weights and
    lays every tensor out in its exact SBUF shape (contiguous DMAs)."""
    x = np.asarray(x, np.float32)
    g_ = np.asarray(ln_gamma, np.float32)
    be = np.asarray(ln_beta, np.float32)
    w_qkv = np.asarray(w_qkv, np.float32)
    b_qkv = np.asarray(b_qkv, np.float32)
    w_proj = np.asarray(w_proj, np.float32)

    def sb_layout(m):  # [K, M] -> [P, K//P, M]
        return np.ascontiguousarray(
            m.reshape(m.shape[0] // P, P, m.shape[1]).transpose(1, 0, 2))

    in_maps = []
    for core in range(8):
        b = core // 2
        gr = core % 2
        rs = slice(gr * HPC * DH, (gr + 1) * HPC * DH)
        wq, wk, wv = (w_qkv[i * C:(i + 1) * C][rs] for i in range(3))
        bq, bk, bv = (b_qkv[i * C:(i + 1) * C][rs] for i in range(3))
        # gamma folds into W columns; beta folds into the bias
        wqg, wkg, wvg = (w * g_[None, :] for w in (wq, wk, wv))
        bq = bq + wq @ be
        bk = bk + wk @ be
        bv = bv + wv @ be
        # x in strip-major SBUF shape [NSTRIP, P, KC, STRIP]
        xs = (x[b].reshape(KC, P, NSTRIP, STRIP).transpose(2, 1, 0, 3))
        in_maps.append({
            "x_sh": np.ascontiguousarray(xs),
            "wqkT": sb_layout(np.concatenate([wqg, wkg], 0).T),
            "wvT": sb_layout(wvg.T),
            "wprojT": sb_layout(w_proj[:, rs].T),
            "bqk": np.ascontiguousarray(
                np.concatenate([bq, bk]).reshape(4, P).T),
            "wsum": np.ascontiguousarray(
                np.concatenate([wqg.sum(1), wkg.sum(1)]).reshape(4, P).T),
            "wvsum": np.ascontiguousarray(wvg.sum(1)[None, :]),
            "bv": np.ascontiguousarray(bv[None, :]),
            "vones": np.ones((P, LCH * HPC), np.float32),
        })
    return in_maps


def combine(partials, x, b_proj):
    out = np.empty((B, C, L), np.float32)
    for b in range(B):
        # partial [NSTRIP, P, 4, STRIP] -> [C, L]
        p = (np.asarray(partials[2 * b]) + np.asarray(partials[2 * b + 1]))
        p = p.transpose(2, 1, 0, 3).reshape(C, L)
        out[b] = p + np.asarray(b_proj, np.float32)[:, None] \
            + np.asarray(x, np.float32)[b]
    return out


def run_cores(in_maps, trace=False, **kw):
    nc = _get_nc()
    return run_bass_kernel_spmd(nc, in_maps, core_ids=list(range(8)),
                                trace=trace, **kw)


def kernel(**inputs):
    in_maps = make_core_inputs(**inputs)
    res = run_cores(in_maps)
    partials = [r["out_part"] for r in res.results]
    return combine(partials, inputs["x"], inputs["b_proj"])


# revision 39
# speedup vs baseline: 1.5107x; 1.5107x over previous
"""AttentionBlock (ChannelNorm + MHA + proj + residual) Trainium2 Bass kernel.

Sharding: 8 cores = 4 batches x 2 head-groups. Core c handles batch c//2 and
heads [4*(c%2), 4*(c%2)+4). Each core computes LayerNorm + its slice of the
QKV projection + attention for its 4 heads + a partial proj_out contraction.
The host sums the two partials per batch and adds proj bias + residual.

All matmuls run in float32r (fast fp32 mode, ~1.5e-4 rel err). The whole
device pipeline works in a channels-on-partitions [C, L] layout so no
transposes are needed anywhere:
  - LN stats (sum, sum-sq over C) via ones-vector matmuls, rstd via
    reciprocal_approx_fast, per-position stats broadcast across partitions
    with the Pool partition_broadcast instruction.
  - q^T,k^T come out of the QKV GEMM as [d, L]; v as [L, d] — exactly the
    layouts the attention matmuls need.
  - scores are computed transposed (s^T[lk, lq]), softmax denominator comes
    free from a ones-column appended to v (M=65 matmul), normalization is
    folded in after the o^T accumulation.

Host pre-lays every tensor out in its exact SBUF shape so each DMA is one
contiguous copy.
"""
import numpy as np

import concourse.bass as bass
import concourse.mybir as mybir
import concourse.tile as tile
from concourse import bacc
from concourse.bass_utils import run_bass_kernel_spmd

F32 = mybir.dt.float32
F32R = mybir.dt.float32r

B, C, L, H = 4, 512, 2048, 8
DH = C // H          # 64
G = 2                # head groups (cores per batch)
HPC = H // G         # 4 heads per core
P = 128
KC = C // P          # 4 contraction chunks
NSTRIP = 4
STRIP = L // NSTRIP  # 512
LCH = L // P         # 16 l-chunks
SCALE = DH ** -0.5
EPS = 1e-5
ALU = mybir.AluOpType
ACTF = mybir.ActivationFunctionType


def build_nc():
    nc = bacc.Bacc()
    x_d = nc.dram_tensor("x_sh", [NSTRIP, P, KC, STRIP], F32R, kind="ExternalInput")
    wqk_d = nc.dram_tensor("wqkT", [P, KC, 2 * HPC * DH], F32R, kind="ExternalInput")
    wv_d = nc.dram_tensor("wvT", [P, KC, HPC * DH], F32R, kind="ExternalInput")
    wp_d = nc.dram_tensor("wprojT", [P, 2, C], F32R, kind="ExternalInput")
    bqk_d = nc.dram_tensor("bqk", [P, 4], F32, kind="ExternalInput")
    bv_d = nc.dram_tensor("bv", [1, HPC * DH], F32, kind="ExternalInput")
    vones_d = nc.dram_tensor("vones", [P, LCH * HPC], F32R, kind="ExternalInput")
    out_d = nc.dram_tensor("out_part", [NSTRIP, P, 4, STRIP], F32,
                           kind="ExternalOutput")
    # DRAM bounce buffers for partition-broadcasts of per-position vectors
    rstd_d = nc.dram_tensor("rstd_scr", [NSTRIP, STRIP], F32)
    murstd_d = nc.dram_tensor("murstd_scr", [NSTRIP, STRIP], F32)
    rz_d = nc.dram_tensor("rz_scr", [HPC, 4, STRIP], F32)

    with tile.TileContext(nc) as tc:
        with (
            tc.tile_pool(name="persist", bufs=1) as pp,
            tc.tile_pool(name="small", bufs=4) as sp,
        ):
            # ---- persistent tiles ----
            ones_sb = pp.tile([P, 1], F32R)
            wqk_sb = pp.tile([P, KC, 2 * HPC * DH], F32R)    # [128,4,512]
            wv_sb = pp.tile([P, KC, HPC * DH], F32R)         # [128,4,256]
            wp_sb = pp.tile([P, 2, C], F32R)                 # [128,2,512]
            bqk_sb = pp.tile([P, 4], F32)
            bvb_sb = pp.tile([P, HPC * DH], F32)             # broadcast v bias
            qkT_sb = pp.tile([P, 4, L], F32R)                # q^T,k^T [c_out,l]
            v_sb = pp.tile([P, LCH, HPC, DH + 1], F32R)      # v + ones col
            onT_sb = pp.tile([P, 2, L], F32R)                # normalized o^T
            eps_sb = sp.tile([NSTRIP, 1], F32)

            nc.sync.dma_start(ones_sb[:], vones_d[:, 0:1])
            nc.vector.memset(eps_sb[:], EPS)

            # ================= phase A: LN stats + QKV GEMMs =================
            with (
                tc.tile_pool(name="xa", bufs=4) as xa,
                tc.tile_pool(name="x2a", bufs=1) as x2a,
                tc.tile_pool(name="stats", bufs=1) as st,
                tc.tile_pool(name="bcst", bufs=2) as bc,
                tc.tile_pool(name="ep2", bufs=3) as ep2,
                tc.tile_pool(name="psumA", bufs=2, space="PSUM") as psA,
            ):
                x_tiles = []
                for s in range(NSTRIP):
                    x_sb = xa.tile([P, KC, STRIP], F32R, tag="x", name=f"x{s}")
                    nc.sync.dma_start(x_sb[:], x_d[s])
                    x_tiles.append(x_sb)
                nc.sync.dma_start(wqk_sb[:], wqk_d[:])
                nc.sync.dma_start(wv_sb[:], wv_d[:])
                nc.sync.dma_start(wp_sb[:], wp_d[:])
                nc.sync.dma_start(bqk_sb[:], bqk_d[:])
                nc.sync.dma_start(bvb_sb[:], bv_d[0:1, :].partition_broadcast(P).opt())
                nc.sync.dma_start(
                    v_sb[:, :, :, DH:DH + 1],
                    vones_d.rearrange("p (lc h one) -> p lc h one", h=HPC, one=1),
                )

                # ---- stats matmuls for every strip (PE never blocks) ----
                sum4_sb = st.tile([NSTRIP, STRIP], F32, tag="sum4")
                sq4_sb = st.tile([NSTRIP, STRIP], F32, tag="sq4")
                for s in range(NSTRIP):
                    x_sb = x_tiles[s]
                    x2 = x2a.tile([P, KC, STRIP], F32R, tag="x2",
                                  name=f"x2_{s}")
                    nc.scalar.activation(x2[:], x_sb.bitcast(F32)[:],
                                         ACTF.Square)
                    ps_sum = psA.tile([1, STRIP], F32, tag="stat_sum",
                                      name=f"psum{s}")
                    ps_sq = psA.tile([1, STRIP], F32, tag="stat_sq",
                                     name=f"psq{s}")
                    for kc in range(KC):
                        nc.tensor.matmul(
                            ps_sum[:], ones_sb[:], x_sb[:, kc, :],
                            start=(kc == 0), stop=(kc == KC - 1),
                        )
                    for kc in range(KC):
                        nc.tensor.matmul(
                            ps_sq[:], ones_sb[:], x2[:, kc, :],
                            start=(kc == 0), stop=(kc == KC - 1),
                        )
                    scr_a = st.tile([1, STRIP], F32, tag="scr_a", bufs=2,
                                    name=f"scr_a{s}")
                    scr_b = st.tile([1, STRIP], F32, tag="scr_b", bufs=2,
                                    name=f"scr_b{s}")
                    nc.vector.tensor_copy(scr_a[:], ps_sum[:])
                    nc.vector.tensor_copy(scr_b[:], ps_sq[:])
                    nc.gpsimd.dma_start(sum4_sb[s:s + 1, :], scr_a[:])
                    nc.gpsimd.dma_start(sq4_sb[s:s + 1, :], scr_b[:])

                # ---- one stats chain for all strips on [4, 512] ----
                mu = st.tile([NSTRIP, STRIP], F32, tag="mu")
                t2 = st.tile([NSTRIP, STRIP], F32, tag="t2")
                var = st.tile([NSTRIP, STRIP], F32, tag="var")
                rstd = st.tile([NSTRIP, STRIP], F32, tag="rstd")
                murstd = st.tile([NSTRIP, STRIP], F32, tag="murstd")
                nc.vector.tensor_scalar_mul(mu[:], sum4_sb[:], 1.0 / C)
                nc.vector.tensor_mul(t2[:], mu[:], mu[:])
                nc.vector.scalar_tensor_tensor(
                    var[:], sq4_sb[:], 1.0 / C, t2[:],
                    op0=ALU.mult, op1=ALU.subtract,
                )
                nc.scalar.activation(var[:], var[:], ACTF.Sqrt, bias=eps_sb[:])
                nc.vector.reciprocal(rstd[:], var[:])
                nc.vector.tensor_mul(murstd[:], mu[:], rstd[:])
                nc.gpsimd.dma_start(rstd_d[:], rstd[:])
                nc.gpsimd.dma_start(murstd_d[:], murstd[:])

                # ---- hn = x*rstd - mu*rstd, then QKV GEMMs ----
                for s in range(NSTRIP):
                    ls = bass.ts(s, STRIP)
                    x_sb = x_tiles[s]
                    xf = x_sb.bitcast(F32)
                    rstd_b = bc.tile([P, STRIP], F32, tag="rstd_b")
                    murstd_b = bc.tile([P, STRIP], F32, tag="murstd_b")
                    nc.gpsimd.dma_start(
                        rstd_b[:], rstd_d[s:s + 1, :].partition_broadcast(P).opt())
                    nc.gpsimd.dma_start(
                        murstd_b[:],
                        murstd_d[s:s + 1, :].partition_broadcast(P).opt())

                    hn = ep2.tile([P, KC, STRIP], F32R, tag="hn", bufs=2)
                    hf = hn.bitcast(F32)
                    for kc in range(KC):
                        nc.gpsimd.tensor_mul(hn[:, kc, :], xf[:, kc, :], rstd_b[:])
                    for kc in range(KC):
                        nc.vector.tensor_sub(hn[:, kc, :], hf[:, kc, :],
                                             murstd_b[:])

                    # q^T,k^T GEMM: out [c_out, l]
                    for mc in range(4):
                        pqk = psA.tile([P, STRIP], F32, tag="qk")
                        for kc in range(KC):
                            nc.tensor.matmul(
                                pqk[:], wqk_sb[:, kc, bass.ts(mc, P)],
                                hn[:, kc, :],
                                start=(kc == 0), stop=(kc == KC - 1),
                            )
                        nc.scalar.activation(
                            qkT_sb[:, mc, ls], pqk[:], ACTF.Identity,
                            bias=bqk_sb[:, mc:mc + 1],
                        )

                    # v GEMM: out [l, d]
                    for lc in range(STRIP // P):
                        lg = s * (STRIP // P) + lc
                        pv = psA.tile([P, HPC * DH], F32, tag="v")
                        for kc in range(KC):
                            nc.tensor.matmul(
                                pv[:], hn[:, kc, bass.ts(lc, P)],
                                wv_sb[:, kc, :],
                                start=(kc == 0), stop=(kc == KC - 1),
                            )
                        nc.vector.tensor_add(
                            v_sb[:, lg, :, 0:DH],
                            pv.rearrange("p (h d) -> p h d", h=HPC),
                            bvb_sb.rearrange("p (h d) -> p h d", h=HPC),
                        )

            # ================= phase B: attention per head =================
            with (
                tc.tile_pool(name="expp", bufs=4) as ep,
                tc.tile_pool(name="rdout", bufs=2) as ro,
                tc.tile_pool(name="psumB", bufs=2, space="PSUM") as psB,
                tc.tile_pool(name="psumO", bufs=4, space="PSUM") as psO,
            ):
                for h in range(HPC):
                    po = (h % 2) * DH
                    qT = qkT_sb[po:po + DH, h // 2, :]
                    kT = qkT_sb[po:po + DH, 2 + h // 2, :]
                    oT = [psO.tile([DH + 1, STRIP], F32, tag="oT", name=f"oT{h}_{i}")
                          for i in range(4)]
                    for lk in range(LCH):
                        ex = ep.tile([P, L], F32R, tag="expT")
                        for half in range(2):
                            pst = psB.tile([P, 1024], F32, tag="sT")
                            for q2 in range(2):
                                nc.tensor.matmul(
                                    pst[:, bass.ts(q2, 512)],
                                    kT[:, bass.ts(lk, P)],
                                    qT[:, bass.ds(half * 1024 + q2 * 512, 512)],
                                    start=True, stop=True,
                                )
                            nc.scalar.activation(
                                ex[:, bass.ts(half, 1024)], pst[:],
                                ACTF.Exp, scale=SCALE,
                            )
                        for s in range(4):
                            nc.tensor.matmul(
                                oT[s][:], v_sb[:, lk, h, :], ex[:, bass.ts(s, STRIP)],
                                start=(lk == 0), stop=(lk == LCH - 1),
                            )
                    # normalize: onT[d, l] = oT[d, l] / Z[l]; one batched recip
                    zrow = ro.tile([4, STRIP], F32, tag="zrow")
                    rz4 = ro.tile([4, STRIP], F32, tag="rz4")
                    for s in range(4):
                        zscr = ro.tile([1, STRIP], F32, tag="zscr",
                                       name=f"zscr{h}_{s}")
                        nc.vector.tensor_copy(zscr[:], oT[s][DH:DH + 1, :])
                        nc.sync.dma_start(zrow[s:s + 1, :], zscr[:])
                    nc.vector.reciprocal(rz4[:], zrow[:])
                    nc.sync.dma_start(rz_d[h], rz4[:])
                    for s in range(4):
                        rz_b = ro.tile([DH, STRIP], F32, tag="rz_b", bufs=4)
                        nc.sync.dma_start(
                            rz_b[:],
                            rz_d[h, s:s + 1, :].partition_broadcast(DH).opt())
                        nc.vector.tensor_mul(
                            onT_sb[po:po + DH, h // 2, bass.ts(s, STRIP)],
                            oT[s][0:DH, :], rz_b[:],
                        )

            # ================= phase C: proj partial =================
            with (
                tc.tile_pool(name="outp", bufs=2) as op_,
                tc.tile_pool(name="psumC", bufs=2, space="PSUM") as psC,
            ):
                for s in range(NSTRIP):
                    ls = bass.ts(s, STRIP)
                    ot = op_.tile([P, 4, STRIP], F32, tag="out")
                    for mc in range(4):
                        ppj = psC.tile([P, STRIP], F32, tag="proj")
                        for kc in range(2):
                            nc.tensor.matmul(
                                ppj[:], wp_sb[:, kc, bass.ts(mc, P)],
                                onT_sb[:, kc, ls],
                                start=(kc == 0), stop=(kc == 1),
                            )
                        nc.vector.tensor_copy(ot[:, mc, :], ppj[:])
                    nc.sync.dma_start(out_d[s], ot[:])

    nc.compile()
    return nc


_NC = None


def _get_nc():
    global _NC
    if _NC is None:
        _NC = build_nc()
    return _NC


def make_core_inputs(x, ln_gamma, ln_beta, w_qkv, b_qkv, w_proj, b_proj):
    """Host-side shard prep. Folds ln_gamma/ln_beta into the QKV weights and
    lays every tensor out in its exact SBUF shape (contiguous DMAs)."""
    x = np.asarray(x, np.float32)
    g_ = np.asarray(ln_gamma, np.float32)
    be = np.asarray(ln_beta, np.float32)
    w_qkv = np.asarray(w_qkv, np.float32)
    b_qkv = np.asarray(b_qkv, np.float32)
    w_proj = np.asarray(w_proj, np.float32)

    def sb_layout(m):  # [K, M] -> [P, K//P, M]
        return np.ascontiguousarray(
            m.reshape(m.shape[0] // P, P, m.shape[1]).transpose(1, 0, 2))

    in_maps = []
    for core in range(8):
        b = core // 2
        gr = core % 2
        rs = slice(gr * HPC * DH, (gr + 1) * HPC * DH)
        wq, wk, wv = (w_qkv[i * C:(i + 1) * C][rs] for i in range(3))
        bq, bk, bv = (b_qkv[i * C:(i + 1) * C][rs] for i in range(3))
        # gamma folds into W columns; beta folds into the bias
        wqg, wkg, wvg = (w * g_[None, :] for w in (wq, wk, wv))
        bq = bq + wq @ be
        bk = bk + wk @ be
        bv = bv + wv @ be
        # x in strip-major SBUF shape [NSTRIP, P, KC, STRIP]
        xs = (x[b].reshape(KC, P, NSTRIP, STRIP).transpose(2, 1, 0, 3))
        in_maps.append({
            "x_sh": np.ascontiguousarray(xs),
            "wqkT": sb_layout(np.concatenate([wqg, wkg], 0).T),
            "wvT": sb_layout(wvg.T),
            "wprojT": sb_layout(w_proj[:, rs].T),
            "bqk": np.ascontiguousarray(
                np.concatenate([bq, bk]).reshape(4, P).T),
            "bv": np.ascontiguousarray(bv[None, :]),
            "vones": np.ones((P, LCH * HPC), np.float32),
        })
    return in_maps


def combine(partials, x, b_proj):
    out = np.empty((B, C, L), np.float32)
    for b in range(B):
        # partial [NSTRIP, P, 4, STRIP] -> [C, L]
        p = (np.asarray(partials[2 * b]) + np.asarray(partials[2 * b + 1]))
        p = p.transpose(2, 1, 0, 3).reshape(C, L)
        out[b] = p + np.asarray(b_proj, np.float32)[:, None] \
            + np.asarray(x, np.float32)[b]
    return out


def run_cores(in_maps, trace=False, **kw):
    nc = _get_nc()
    return run_bass_kernel_spmd(nc, in_maps, core_ids=list(range(8)),
                                trace=trace, **kw)


def kernel(**inputs):
    in_maps = make_core_inputs(**inputs)
    res = run_cores(in_maps)
    partials = [r["out_part"] for r in res.results]
    return combine(partials, inputs["x"], inputs["b_proj"])
